# revision 19
# baseline (speedup 1.0000x reference)
"""Single-head causal attention (S=2048, B=8, D=1024) for 8 TRN2 NeuronCores.

Sharding: data-parallel over the batch dim — core c computes batch element c.

Per-core Bass/Tile kernel layout (bf16 matmul operands, fp32 PSUM accum):
  - Host passes query/key/value pre-transposed to [D, S] in bf16 so every
    matmul contraction dim lands on SBUF partitions without on-chip
    transposes.
  - The Q projection is folded into the key side (B = SCALE * Wk^T Wq) and
    the output projection into the value side (C = Wv^T Wo^T), so only two
    D x D projections run on-device.
  - G^T = (kin @ B)^T is kept SBUF-resident in bf16 (32KB/partition) — no
    DRAM round-trip for the projected keys.
  - The K projection runs di-outer in two m-halves so the first matmul
    needs only one 128-row weight strip and one input slice (~0.26MB of
    DMA), cutting the head stall; critical head DMAs are spread across
    all four engine queues.
  - Scores are computed transposed ([j, i] = keys on partitions), which
    makes exp() a straight ScalarE pass out of PSUM and feeds the PV
    matmul with no on-chip transpose of the attention matrix.
  - Softmax skips the max-subtraction (scores are ~N(0,1); exp cannot
    overflow) and gets the denominator from a 1-wide all-ones rider on
    the PV accumulation. Each PV block runs denominator-chain, then dh0,
    then dh1, so the reciprocal and the dh0 epilogue hide under the dh1
    matmuls (shortens the kernel tail).
  - bf16 matmuls run at 1 cyc/row at any width, so the causal band tiles
    are cut exactly at 128 granularity (no fp32r min-256 constraint).
  - Output is stored bf16 and upcast on the host (rel-err budget 2e-2;
    bf16 end-to-end lands ~5e-3).
  - Variants with the "_zb" suffix skip all bias handling (this problem's
    biases are zero vectors).
"""

import math
from contextlib import ExitStack

import numpy as np
import ml_dtypes

import concourse.bass as bass
import concourse.mybir as mybir
import concourse.tile as tile
from concourse import bacc
from concourse.bass_utils import run_bass_kernel_spmd

S, B, D = 2048, 8, 1024
P = 128
DI = D // P  # 8 contraction chunks
JC = S // P  # 16 key chunks
NSB = 4  # query superblocks
SBW = S // NSB  # 512 queries per superblock
SCALE = 1.0 / math.sqrt(D)
CORES = list(range(8))
F32 = mybir.dt.float32
BF16 = mybir.dt.bfloat16
BF16NP = ml_dtypes.bfloat16


_cache: dict[str, object] = {}


def _build(variant: str):
    """variant: 'causal' (skip masked tiles), 'full' (no mask), 'masked'
    (arbitrary 0/1 mask streamed from DRAM); '_zb' suffix = biases all zero."""
    zb = variant.endswith("_zb")
    mv = variant[:-3] if zb else variant
    assert mv in ("causal", "full", "masked")
    nc = bacc.Bacc("TRN2", num_devices=len(CORES))

    qin = nc.dram_tensor("qin", [D, S], BF16, kind="ExternalInput").ap()
    kin = nc.dram_tensor("kin", [D, S], BF16, kind="ExternalInput").ap()
    vin = nc.dram_tensor("vin", [D, S], BF16, kind="ExternalInput").ap()
    wkt = nc.dram_tensor("wkt", [D, D], BF16, kind="ExternalInput").ap()
    wvt = nc.dram_tensor("wvt", [D, D], BF16, kind="ExternalInput").ap()
    onesd = nc.dram_tensor("onesd", [P, 1], BF16, kind="ExternalInput").ap()
    if not zb:
        wvec = nc.dram_tensor("wvec", [P, JC], F32, kind="ExternalInput").ap()
        borep = nc.dram_tensor("borep", [P, D], F32, kind="ExternalInput").ap()
    if mv == "masked":
        maskt = nc.dram_tensor("maskt", [S, S], BF16, kind="ExternalInput").ap()
    out = nc.dram_tensor("out", [S, D], BF16, kind="ExternalOutput").ap()

    def nj(sb):
        return 4 * sb + 4 if mv == "causal" else JC

    with tile.TileContext(nc) as tc, ExitStack() as ctx:
        pool_const = ctx.enter_context(tc.tile_pool(name="const", bufs=1))
        pool_g = ctx.enter_context(tc.tile_pool(name="gres", bufs=1))
        pool_v = ctx.enter_context(tc.tile_pool(name="vres", bufs=1))
        pool_qt = ctx.enter_context(tc.tile_pool(name="qtp", bufs=2))
        pool_pt = ctx.enter_context(tc.tile_pool(name="ptp", bufs=2))
        pool_w = ctx.enter_context(tc.tile_pool(name="wts", bufs=3))
        pool_in = ctx.enter_context(tc.tile_pool(name="ins", bufs=2))
        pool_y = ctx.enter_context(tc.tile_pool(name="yp", bufs=4))
        pool_small = ctx.enter_context(tc.tile_pool(name="smal", bufs=2))
        pool_mask = ctx.enter_context(tc.tile_pool(name="mskp", bufs=2))
        psum_pp = ctx.enter_context(tc.tile_pool(name="pps", bufs=4, space="PSUM"))
        psum_qk = ctx.enter_context(tc.tile_pool(name="qkps", bufs=3, space="PSUM"))
        psum_l = ctx.enter_context(tc.tile_pool(name="lps", bufs=1, space="PSUM"))

        ones_t = pool_const.tile([P, 1], BF16)
        if not zb:
            wv_t = pool_const.tile([P, JC], F32)
            borep_t = pool_const.tile([P, D], F32)

        # SBUF-resident projected keys G^T[g, j] and values V' = v @ C
        g_sb = pool_g.tile([P, DI, S], BF16)
        v_sb = pool_v.tile([P, JC, D], BF16)

        qt_tiles = {}

        def emit_qt_prefetch(sb, queue):
            qt = pool_qt.tile([P, DI, SBW], BF16, tag="qt", name=f"qt{sb}")
            queue.dma_start(
                qt[:],
                qin.rearrange("(di p) s -> p di s", p=P)[
                    :, :, sb * SBW : (sb + 1) * SBW
                ],
            )
            qt_tiles[sb] = qt

        wkr = wkt.rearrange("(di p) o -> p di o", p=P)
        wvr = wvt.rearrange("(di p) o -> p di o", p=P)
        kr = kin.rearrange("(di p) s -> p di s", p=P)
        vr = vin.rearrange("(di p) s -> p di s", p=P)

        # ---- head: critical loads fanned over all four engine queues ----
        # wk is loaded as per-(di, half) strips [P, 512]; the K projection
        # consumes (di=0, half=0) first. kin's first block loads per-di.
        wk0 = pool_w.tile([P, DI, 512], BF16, tag="wt", name="wk0")
        wk1 = pool_w.tile([P, DI, 512], BF16, tag="wt", name="wk1")
        wk_h = [wk0, wk1]
        tin0 = pool_in.tile([P, DI, 512], BF16, tag="tin", name="tin_k0")

        # The DMA device drains triggers in arrival order at ~364ns per
        # 128KB strip, so trigger order IS data order. Pool (fires from
        # t~100) carries all half-0 strips; SP interleaves the kin slices;
        # half-1 strips trail on both queues; ACT stays silent at the head
        # (its queue opens late behind LoadActFuncSet, and any big transfer
        # here would push the critical strips back).
        # both first-matmul operands go on Pool's SWDGE queue (44ns descriptor
        # gen vs SP's 625ns HWDGE overhead), so the second transfer starts
        # the moment the first finishes
        nc.gpsimd.dma_start(wk0[:, 0, :], wkr[:, 0, 0:512])
        nc.gpsimd.dma_start(tin0[:, 0, :], kr[:, 0, 0:512])
        for di in range(1, DI):
            nc.gpsimd.dma_start(wk0[:, di, :], wkr[:, di, 0:512])
            nc.sync.dma_start(tin0[:, di, :], kr[:, di, 0:512])
        for di in range(DI):
            q = nc.gpsimd if di % 2 == 0 else nc.sync
            q.dma_start(wk1[:, di, :], wkr[:, di, 512:1024])

        # non-critical loads ride strictly behind the head
        nc.gpsimd.dma_start(ones_t[:], onesd[:])
        emit_qt_prefetch(0, nc.gpsimd)
        if not zb:
            nc.gpsimd.dma_start(wv_t[:], wvec[:])
            nc.gpsimd.dma_start(borep_t[:], borep[:])

        # ---------------- K projection (di-outer, two m-halves) ----------
        for jc4 in range(S // 512):
            if jc4 == 0:
                tin = tin0
            else:
                tin = pool_in.tile([P, DI, 512], BF16, tag="tin")
                nc.sync.dma_start(tin[:], kr[:, :, jc4 * 512 : (jc4 + 1) * 512])
            for h in range(2):
                pss = [
                    psum_pp.tile([P, 512], F32, tag="ps", name=f"kp{jc4}_{h}_{m}")
                    for m in range(4)
                ]
                for di in range(DI):
                    for m in range(4):
                        nc.tensor.matmul(
                            pss[m][:],
                            wk_h[h][:, di, m * P : (m + 1) * P],
                            tin[:, di, :],
                            start=di == 0,
                            stop=di == DI - 1,
                        )
                for m in range(4):
                    nc.scalar.copy(
                        g_sb[:, 4 * h + m, jc4 * 512 : (jc4 + 1) * 512], pss[m][:]
                    )
            # weight/value loads staggered through the projection
            if jc4 == 0:
                wv0 = pool_w.tile([P, DI, 512], BF16, tag="wt", name="wv0")
                nc.scalar.dma_start(wv0[:], wvr[:, :, 0:512])
                emit_qt_prefetch(1, nc.gpsimd)
            elif jc4 == 1:
                wv1 = pool_w.tile([P, DI, 512], BF16, tag="wt", name="wv1")
                nc.scalar.dma_start(wv1[:], wvr[:, :, 512:1024])
            elif jc4 == 2:
                vtin0 = pool_in.tile([P, DI, 512], BF16, tag="tin", name="tin_v0")
                nc.sync.dma_start(vtin0[:], vr[:, :, 0:512])
        wv_h = [wv0, wv1]

        # ---------------- attention emitters ----------------
        def emit_qk(sb):
            n = nj(sb)
            qt = qt_tiles[sb]
            pt = pool_pt.tile([P, JC, SBW], BF16, tag="pt", name=f"pt{sb}")
            for jc in range(n):
                # causal: queries below the diagonal band are all-masked;
                # bf16 runs full-rate at any width, so cut exactly.
                off = max(0, (jc - 4 * sb) * P) if mv == "causal" else 0
                ps = psum_qk.tile([P, SBW], F32, tag="ps", name=f"qk{sb}_{jc}")
                for di in range(DI):
                    nc.tensor.matmul(
                        ps[:, off:],
                        g_sb[:, di, jc * P : (jc + 1) * P],
                        qt[:, di, off:],
                        start=di == 0,
                        stop=di == DI - 1,
                    )
                nc.scalar.activation(
                    pt[:, jc, off:],
                    ps[:, off:],
                    mybir.ActivationFunctionType.Exp,
                    bias=0.0 if zb else wv_t[:, jc : jc + 1],
                )
                if mv == "causal" and jc >= 4 * sb:
                    # zero the j > i triangle in the diagonal chunk
                    bend = min(off + P, SBW)
                    nc.gpsimd.affine_select(
                        out=pt[:, jc, off:bend],
                        in_=pt[:, jc, off:bend],
                        compare_op=mybir.AluOpType.is_ge,
                        fill=0.0,
                        base=sb * SBW - jc * P + off,
                        pattern=[[1, bend - off]],
                        channel_multiplier=-1,
                    )
                if mv == "masked":
                    mtile = pool_mask.tile([P, SBW], BF16, tag="mt")
                    nc.sync.dma_start(
                        mtile[:],
                        maskt[jc * P : (jc + 1) * P, sb * SBW : (sb + 1) * SBW],
                    )
                    nc.vector.tensor_mul(pt[:, jc, :], pt[:, jc, :], mtile[:])
            return pt

        def emit_out(sb, pt):
            # Fused PV + denominator: lhsT = p^T chunks, rhs = V' in [j, g]
            # layout, so accumulation lands as y[i, g] with queries on
            # partitions. Denominator chain runs first, then dh0, then dh1,
            # so recip/mul/store of earlier pieces hide under later matmuls.
            for ic in range(SBW // P):
                njc = 4 * sb + ic + 1 if mv == "causal" else nj(sb)
                l_ps = psum_l.tile([P, 32], F32, tag="lps", name=f"l{sb}_{ic}")
                for jc in range(njc):
                    nc.tensor.matmul(
                        l_ps[:, :1],
                        pt[:, jc, ic * P : (ic + 1) * P],
                        ones_t[:, :1],
                        start=jc == 0,
                        stop=jc == njc - 1,
                    )
                rinv = pool_small.tile([P, 1], F32, tag="rinv", name=f"ri{sb}_{ic}")
                nc.vector.reciprocal(rinv[:], l_ps[:, 0:1])
                # the very last output block ends on a narrow piece so the
                # post-PE epilogue chain (mul -> store -> drain) is short
                last_block = sb == NSB - 1 and ic == SBW // P - 1
                pieces = [(0, 512), (512, 896), (896, 1024)] if last_block else [
                    (0, 512), (512, 1024)]
                for pi, (c0, c1) in enumerate(pieces):
                    yps = psum_pp.tile(
                        [P, c1 - c0], F32, tag="ps", name=f"y{sb}_{ic}_{pi}"
                    )
                    for jc in range(njc):
                        nc.tensor.matmul(
                            yps[:],
                            pt[:, jc, ic * P : (ic + 1) * P],
                            v_sb[:, jc, c0:c1],
                            start=jc == 0,
                            stop=jc == njc - 1,
                        )
                    ysb = pool_y.tile(
                        [P, c1 - c0], BF16, tag="y", name=f"ysb{sb}_{ic}_{pi}"
                    )
                    nc.scalar.mul(ysb[:], yps[:], rinv[:])
                    if not zb:
                        nc.vector.tensor_add(
                            ysb[:], ysb[:], borep_t[:, c0:c1]
                        )
                    # the very last store issues from the ACT queue: it then
                    # follows its mul in-order with no cross-engine sem hop
                    oq = nc.scalar if last_block and pi == len(pieces) - 1 else nc.sync
                    oq.dma_start(
                        out[
                            sb * SBW + ic * P : sb * SBW + (ic + 1) * P,
                            c0:c1,
                        ],
                        ysb[:],
                    )

        # QK(0) right after the K projection: it only needs g_sb and qt0,
        # so the PE never waits on the V projection's trailing copies.
        pt0 = emit_qk(0)

        # ---------------- V projection ----------------
        for jc4 in range(S // 512):
            if jc4 == 0:
                tin = vtin0
            else:
                tin = pool_in.tile([P, DI, 512], BF16, tag="tin")
                nc.sync.dma_start(tin[:], vr[:, :, jc4 * 512 : (jc4 + 1) * 512])
            for jb in range(512 // P):
                jg = jc4 * 4 + jb
                for nn in range(D // 512):
                    ps = psum_pp.tile([P, 512], F32, tag="ps")
                    for di in range(DI):
                        nc.tensor.matmul(
                            ps[:],
                            tin[:, di, jb * P : (jb + 1) * P],
                            wv_h[nn][:, di, :],
                            start=di == 0,
                            stop=di == DI - 1,
                        )
                    nc.vector.tensor_copy(
                        v_sb[:, jg, nn * 512 : (nn + 1) * 512], ps[:]
                    )
            if jc4 == 0:
                emit_qt_prefetch(2, nc.gpsimd)

        emit_out(0, pt0)
        for sb in range(1, NSB):
            pt = emit_qk(sb)
            if sb == 1:
                emit_qt_prefetch(3, nc.sync)
            emit_out(sb, pt)

    nc.compile()
    return nc


def _get_nc(variant: str):
    if variant not in _cache:
        _cache[variant] = _build(variant)
    return _cache[variant]


def _detect_variant(mask: np.ndarray) -> str:
    m = np.asarray(mask)[:, :, 0] != 0
    if m.all():
        return "full"
    if np.array_equal(m, np.tril(np.ones((S, S), dtype=bool))):
        return "causal"
    return "masked"


def _full_variant(mask, bq, bv, bo) -> str:
    v = _detect_variant(mask)
    if not (np.any(bq) or np.any(bv) or np.any(bo)):
        v += "_zb"
    return v


def _host_inputs(variant, query, key, value, mask, Wq, bq, Wk, bk, Wv, bv, Wo, bo, c):
    """Per-core device input map (host does layout prep: transposes, SCALE
    and bias folding, bf16 downcast)."""
    zb = variant.endswith("_zb")
    mv = variant[:-3] if zb else variant
    m = {
        "qin": np.ascontiguousarray(query[:, c, :].T).astype(BF16NP),
        "kin": np.ascontiguousarray(key[:, c, :].T).astype(BF16NP),
        "vin": np.ascontiguousarray(value[:, c, :].T).astype(BF16NP),
        # B = SCALE * Wk^T @ Wq: the Q projection is folded into the key
        # side (scores^T = (kin B) @ qin^T against raw queries). Per-query
        # bias terms cancel in softmax; the per-key cross term
        # (key @ Wk.T @ bq) survives and rides the exp bias (wvec).
        "wkt": (SCALE * (Wk.T.astype(np.float64) @ Wq.astype(np.float64))).astype(
            BF16NP
        ),
        # C = Wv^T @ Wo^T: the output projection is folded into V, so the
        # attention-weighted sum lands directly in output space.
        "wvt": (Wv.T.astype(np.float64) @ Wo.T.astype(np.float64)).astype(BF16NP),
        "onesd": np.ones((P, 1), dtype=BF16NP),
    }
    if not zb:
        bo_eff = (bo + Wo @ bv).astype(np.float32)
        m["wvec"] = np.ascontiguousarray(
            (SCALE * (key[:, c, :] @ (Wk.T @ bq))).reshape(JC, P).T
        ).astype(np.float32)
        m["borep"] = np.ascontiguousarray(np.broadcast_to(bo_eff, (P, D)))
    if mv == "masked":
        m["maskt"] = np.ascontiguousarray(
            (np.asarray(mask)[:, :, 0] != 0).T.astype(BF16NP)
        )
    return m


def kernel(query, key, value, mask, Wq, bq, Wk, bk, Wv, bv, Wo, bo):
    query = np.asarray(query, dtype=np.float32)
    key = np.asarray(key, dtype=np.float32)
    value = np.asarray(value, dtype=np.float32)
    Wq = np.asarray(Wq, dtype=np.float32)
    Wk = np.asarray(Wk, dtype=np.float32)
    Wv = np.asarray(Wv, dtype=np.float32)
    Wo = np.asarray(Wo, dtype=np.float32)
    bq = np.asarray(bq, dtype=np.float32)
    bk = np.asarray(bk, dtype=np.float32)
    bv = np.asarray(bv, dtype=np.float32)
    bo = np.asarray(bo, dtype=np.float32)

    variant = _full_variant(mask, bq, bv, bo)
    nc = _get_nc(variant)
    in_maps = [
        _host_inputs(variant, query, key, value, mask, Wq, bq, Wk, bk, Wv, bv, Wo, bo, c)
        for c in CORES
    ]
    res = run_bass_kernel_spmd(nc, in_maps, core_ids=CORES)

    result = np.empty((S, B, D), dtype=np.float32)
    for c in CORES:
        result[:, c, :] = np.asarray(res.results[c]["out"], dtype=np.float32)
    return result
